# revision 1
# baseline (speedup 1.0000x reference)
"""Trainium2 Bass kernel for nn_BasicS2ConvV2.

out[b,d,p,r] = sum_{c,k,a} Wfull[d,c,r,k,a] * x[b,c,k,p,a]
with Wfull gathered on host from the 36 free params (tiny), and the
31.4 GFLOP contraction run on 8 NeuronCores, data-parallel over b.

Per-core device problem: o[p=4096, dr=192] = xs[cka, p]^T @ WT[cka, dr]
with cka = 16*13*12 = 2496 padded to 2560 = 20 k-tiles of 128.

Transposed mapping (vs W-stationary): the x k-tile [128k x 128p] is the
stationary operand and the W k-tile [128k x 192dr] is the moving one, so
every matmul streams 192 columns with the full 128x128 array utilized:
32 p-tiles x 20 k-tiles x 192 = 122,880 PE cycles/core vs 163,840 for
the W-stationary schedule (dr=192 forces a half-empty 64-wide pass).

x ships as fp8 e3m4 (4 mantissa bits) and W as bf16 (mixed-dtype matmul
verified exact on HW); PSUM accumulates fp32. NBF k-tiles of x can be
kept in bf16 for extra accuracy margin if needed (NBF=0: measured rel
err ~1.3e-2 vs the 2e-2 gate). fp8 keeps the kernel PE-bound rather
than DMA-bound.

DMAs are batched G p-tiles per transfer: descriptor generation
(HWDGE) costs ~630ns per DMA instruction regardless of size, so fewer,
larger transfers keep the DGE off the critical path. The first group
(G0 p-tiles) is pt-granular: xp0 issues via gpsimd/SWDGE and the four
wt chunks stripe across both HWDGE engines, which forces the bus order
[xp0, wt0..wt3, xp1..] so pt0's chain never waits long on weights and
the later chains cover the first steady group's arrival. Dummy warmup
matmuls on scratch data burn the PE p-state ramp window during the
initial DMA fill. The last group's leading p-tiles drain early and its
final p-tile runs as two dr-half chains, so the tail is one small
copy+DMA.

Host pre-layout makes every device DMA a fully sequential HBM stream
(partition-major so any p-tile range slices contiguously):
  xs8: [128, NPT, T8, 128] fp8
  xsb: [128, NPT, NBF, 128] bf16 (only if NBF > 0)
  wt:  [128, KT, DR] bf16        (4 chunked DMAs, loaded once)
  o:   [128, NPT, DR] f16
"""

import numpy as np
import ml_dtypes

B, C, KS, P, A = 8, 16, 13, 4096, 12
D, R = 16, 12
CKA = C * KS * A          # 2496
KT = 20                   # contraction tiles of 128 (2560 padded)
CKA_PAD = KT * 128
DR = D * R                # 192
NPT = P // 128            # 32 p-tiles
NBF = 0                   # k-tiles kept in bf16 (accuracy headroom knob)
T8 = KT - NBF             # k-tiles shipped as fp8 e3m4
G = 2                     # p-tiles per steady DMA group
G0 = 4                    # pt-granular first group (covers the g1 arrival)
# groups: (start_pt, n_pt) — first group pt-granular, then 4s, tail of 2
GROUPS = [(0, G0)] + [(s, min(G, NPT - s)) for s in range(G0, NPT, G)]
NWARM = 14                # PE warmup dummy matmuls

MMDT = "mixed"            # kept for test.py compat

_cache = {}


def _emit_body(nc, xs8, xsb, wtile, wt, o, o_dt, xpool, opool, pools, reps,
               do_dma=True, do_mm=True, do_out=True, xt_static=None,
               wt_whole=False):
    import concourse.mybir as mybir

    pspool, phpool = pools

    seq = [(r, gi) for r in range(reps) for gi in range(len(GROUPS))]
    for r, gi in seq:
        p0, n = GROUPS[gi]
        first = (r, gi) == (0, 0)
        last = (r, gi) == seq[-1]
        if do_dma:
            xt8 = xpool.tile([128, n, T8, 128], mybir.dt.float8e3, tag="xt8")
            xtb = (xpool.tile([128, n, NBF, 128], mybir.dt.bfloat16, tag="xtb")
                   if NBF else None)
            if first and not wt_whole:
                # pt-granular first fills, striped across both HWDGE
                # engines so all four wt chunks finish generating early
                # and land on the bus before xp1+ (pt0 never stalls on wt)
                nc.gpsimd.dma_start(xt8[:, 0], xs8[:, p0])
                nc.sync.dma_start(wtile[:, 0:5, :], wt[:, 0:5, :])
                nc.scalar.dma_start(xt8[:, 1], xs8[:, p0 + 1])
                nc.sync.dma_start(wtile[:, 5:10, :], wt[:, 5:10, :])
                nc.scalar.dma_start(wtile[:, 10:15, :], wt[:, 10:15, :])
                nc.sync.dma_start(wtile[:, 15:20, :], wt[:, 15:20, :])
                nc.scalar.dma_start(xt8[:, 2], xs8[:, p0 + 2])
                nc.sync.dma_start(xt8[:, 3], xs8[:, p0 + 3])
                if NBF:
                    for j in range(n):
                        nc.scalar.dma_start(xtb[:, j], xsb[:, p0 + j])
            else:
                nc.scalar.dma_start(xt8[:], xs8[:, p0:p0 + n])
                if NBF:
                    nc.scalar.dma_start(xtb[:], xsb[:, p0:p0 + n])
        else:
            xt8, xtb = xt_static
        if not do_mm:
            continue
        ot = opool.tile([128, n, DR], o_dt, tag="ot")
        if first and do_mm and T8 == KT and do_dma:
            # interleave chains pt0/pt1 by k-segment: xp1 rides the bus
            # ahead of wt1-3 and both chains advance as wt trickles in.
            # Their psum tiles borrow the phpool banks (idle until the
            # tail) so the steady-state ps tag cycling is untouched.
            ps01 = [phpool.tile([128, DR], mybir.dt.float32, tag="ph",
                                name=f"ps01_{jj}") for jj in range(2)]
            for s in range(4):
                for j in range(2):
                    for t in range(s * 5, s * 5 + 5):
                        nc.tensor.matmul(ps01[j][:], xt8[:, j, t, :],
                                         wtile[:, t, :],
                                         start=(t == 0), stop=(t == KT - 1))
            if do_out:
                for j in range(2):
                    nc.vector.tensor_copy(ot[:, j, :], ps01[j][:])
        for j in range(n):
            if first and do_mm and T8 == KT and do_dma and j < 2:
                continue
            if last and j == n - 1:
                # final p-tile: two dr-half chains so the first half's
                # copy+DMA overlap the second half's matmuls, shrinking
                # the tail to one small copy+DMA
                for h in range(2):
                    ph = phpool.tile([128, DR // 2], mybir.dt.float32,
                                     tag="ph")
                    lo, hi = h * (DR // 2), (h + 1) * (DR // 2)
                    for t in range(T8):
                        nc.tensor.matmul(ph[:], xt8[:, j, t, :],
                                         wtile[:, t, lo:hi],
                                         start=(t == 0), stop=(t == KT - 1))
                    for i in range(NBF):
                        t = T8 + i
                        nc.tensor.matmul(ph[:], xtb[:, j, i, :],
                                         wtile[:, t, lo:hi],
                                         start=(t == 0), stop=(t == KT - 1))
                    if do_out:
                        nc.vector.tensor_copy(ot[:, j, lo:hi], ph[:])
                        nc.sync.dma_start(o[:, p0 + j, lo:hi],
                                          ot[:, j, lo:hi])
                continue
            ps = pspool.tile([128, DR], mybir.dt.float32, tag="ps")
            for t in range(T8):
                nc.tensor.matmul(ps[:], xt8[:, j, t, :], wtile[:, t, :],
                                 start=(t == 0), stop=(t == KT - 1))
            for i in range(NBF):
                t = T8 + i
                nc.tensor.matmul(ps[:], xtb[:, j, i, :], wtile[:, t, :],
                                 start=(t == 0), stop=(t == KT - 1))
            if do_out:
                nc.vector.tensor_copy(ot[:, j, :], ps[:])
                if last and j == n - 2:
                    # drain the leading p-tiles early to shorten the tail
                    nc.sync.dma_start(o[:, p0:p0 + n - 1], ot[:, 0:n - 1])
        if do_out and not last:
            nc.sync.dma_start(o[:, p0:p0 + n], ot[:])


def _build_program(mmdt=None, reps=1, loop_n=0, do_dma=True, do_mm=True,
                   do_out=True, internal_io=False):
    import concourse.bacc as bacc
    import concourse.mybir as mybir
    from concourse.tile import TileContext
    from contextlib import nullcontext

    f8 = mybir.dt.float8e3
    bf = mybir.dt.bfloat16
    o_dt = mybir.dt.float16
    nbf = max(NBF, 1)  # dram decl needs a nonzero dim; unused when NBF=0

    nc = bacc.Bacc("TRN2", target_bir_lowering=False, debug=False)
    if internal_io:
        # Timing-probe build: no host I/O traffic; data is device garbage.
        xs8 = nc.dram_tensor("xs8", [128, NPT, T8, 128], f8).ap()
        xsb = nc.dram_tensor("xsb", [128, NPT, nbf, 128], bf).ap()
        wt = nc.dram_tensor("wt", [128, KT, DR], bf).ap()
        o = nc.dram_tensor("o", [128, NPT, DR], o_dt).ap()
        dume = nc.declare_dram_parameter(
            "dume", [1, 8], mybir.dt.float32, isOutput=True)
    else:
        xs8 = nc.declare_dram_parameter(
            "xs8", [128, NPT, T8, 128], f8, isOutput=False)
        xsb = (nc.declare_dram_parameter(
            "xsb", [128, NPT, nbf, 128], bf, isOutput=False)
            if NBF else None)
        wt = nc.declare_dram_parameter("wt", [128, KT, DR], bf, isOutput=False)
        o = nc.declare_dram_parameter(
            "o", [128, NPT, DR], o_dt, isOutput=True)

    with TileContext(nc) as tc:
        with (
            tc.tile_pool(name="wpool", bufs=1) as wpool,
            tc.tile_pool(name="xpool", bufs=3) as xpool,
            tc.tile_pool(name="opool", bufs=3) as opool,
            tc.tile_pool(name="pspool", bufs=4, space="PSUM") as pspool,
            tc.tile_pool(name="phpool", bufs=2, space="PSUM") as phpool,
            tc.tile_pool(name="wmpool", bufs=1, space="PSUM") as wmpool,
        ):
            wtile = wpool.tile([128, KT, DR], bf)

            # Warmup: dummy matmuls on scratch data occupy the PE during the
            # initial DMA fill so the p-state ramp window burns on
            # otherwise-idle time. Results land in a scratch PSUM bank and
            # are never read.
            if NWARM and do_mm:
                scr = wpool.tile([128, 256], bf, tag="scr")
                nc.vector.memset(scr[:], 0.25)
                psw = wmpool.tile([128, 256], mybir.dt.float32, tag="wm")
                for _ in range(NWARM):
                    nc.tensor.matmul(psw[:], scr[:, 0:128], scr[:],
                                     start=True, stop=True)

            xt_static = None
            wt_whole = bool(loop_n) or not do_dma
            if wt_whole:
                nc.sync.dma_start(wtile[:], wt[:])
            if not do_dma:
                x8s = wpool.tile([128, G0, T8, 128], f8, tag="x8s")
                xbs = wpool.tile([128, G0, nbf, 128], bf, tag="xbs")
                nc.any.memset(x8s[:], 0.25)
                nc.any.memset(xbs[:], 0.25)
                xt_static = (x8s, xbs)

            loop_cm = tc.For_i(0, loop_n, 1) if loop_n else nullcontext()
            with loop_cm:
                _emit_body(nc, xs8, xsb, wtile, wt, o, o_dt,
                           xpool, opool, (pspool, phpool), reps,
                           do_dma=do_dma, do_mm=do_mm, do_out=do_out,
                           xt_static=xt_static, wt_whole=wt_whole)

            if internal_io:
                dtile = opool.tile([1, 8], mybir.dt.float32, tag="dume")
                nc.any.memset(dtile[:], 1.0)
                nc.sync.dma_start(dume[:], dtile[:])

    nc.compile()
    return nc


def _get_program():
    if "main" not in _cache:
        _cache["main"] = _build_program()
    return _cache["main"]


def _prep_inputs(x, W, idx_map, tivr, tir):
    """Host prep: weight gather + relayout to sequential-DMA order."""
    Wm = W[:, :, idx_map].reshape(D, C, KS, A)
    Wfull = Wm[:, :, tivr[:, :, None], tir[:, None, :]]       # [d,c,r,k,a]
    WT = Wfull.transpose(1, 3, 4, 0, 2).reshape(CKA, DR)      # [(c,k,a),(d,r)]
    WT_pad = np.zeros((CKA_PAD, DR), dtype=np.float32)
    WT_pad[:CKA] = WT
    # [2560, DR] -> [128(q), KT(t), DR]
    wt_q = np.ascontiguousarray(
        WT_pad.reshape(KT, 128, DR).transpose(1, 0, 2)).astype(ml_dtypes.bfloat16)

    # x[b,c,k,p,a] -> [b, (c,k,a), p] -> [b, 128(q), NPT(pt), KT(t), 128(p)]
    xt = np.ascontiguousarray(x.transpose(0, 1, 2, 4, 3)).reshape(B, CKA, P)
    xs_pad = np.zeros((B, CKA_PAD, P), dtype=np.float32)
    xs_pad[:, :CKA] = xt
    xs_q = np.ascontiguousarray(
        xs_pad.reshape(B, KT, 128, NPT, 128).transpose(0, 2, 3, 1, 4))
    xs8 = np.ascontiguousarray(
        xs_q[:, :, :, :T8]).astype(ml_dtypes.float8_e3m4)
    xsb = (np.ascontiguousarray(
        xs_q[:, :, :, T8:]).astype(ml_dtypes.bfloat16) if NBF else None)
    return xs8, xsb, wt_q


def kernel(x, W, idx_map, trace_idxv_rot, trace_idx_rot):
    from concourse.bass_utils import run_bass_kernel_spmd

    x = np.asarray(x)
    W = np.asarray(W, dtype=np.float32)
    idx_map = np.asarray(idx_map)
    tivr = np.asarray(trace_idxv_rot)
    tir = np.asarray(trace_idx_rot)

    xs8, xsb, wt_q = _prep_inputs(x, W, idx_map, tivr, tir)

    nc = _get_program()
    in_maps = [{"xs8": xs8[b], "wt": wt_q} for b in range(B)]
    if NBF:
        for b in range(B):
            in_maps[b]["xsb"] = xsb[b]
    res = run_bass_kernel_spmd(nc, in_maps, list(range(B)))

    out = np.empty((B, D, P, R), dtype=np.float32)
    for b in range(B):
        oraw = res.results[b]["o"].astype(np.float32)  # [128, NPT, DR]
        ob = oraw.transpose(1, 0, 2).reshape(P, D, R)
        out[b] = ob.transpose(1, 0, 2)
    return out

